# revision 13
# baseline (speedup 1.0000x reference)
"""DisMax loss first part: logits = -(|s|*d + mean_c(|s|*d)) / temp, where
d[b,c] = ||fn_b - pn_c|| / sqrt(2) = sqrt(1 - cos(f_b, p_c)) for l2-normalized rows.

Data-parallel over the batch across 8 NeuronCores; each core computes
[1024, 512] x [512, 10000] with all numerics on device (host does layout
transposes/slicing and the final bf16->f32 upcast only).

v4: sharded prototype norms + AllGather.
  - Prototype inv-norms are identical work on every core, and v3's trace
    showed the square+reduce+normalize prep saturating DVE/GPSIMD for
    ~150us. Now each core computes inv-norm^2 for only its C/8 = 1250
    classes (from a small host-sliced `ptsl` input), publishes the [1,1250]
    f32 row through a DRAM AllGather (~5KB, latency hides under the 20MB
    prototype DMA), and every core reads back the full [1,10000] row.
  - Per segment: tiny row DMA + bf16 cast + ones-broadcast matmul + one
    fused ACT Sqrt-copy (PSUM->SBUF) makes the [128,1000] inv-norm
    operand; normalize multiplies are split DVE/GPSIMD.
  - Main GEMM: fp8 DoubleRow, 2000-col PSUM groups, software-pipelined
    rounds with a 3-tile window so the PE never idles past the ~3.4us HAM
    window; final affine on DVE tensor_scalar (bf16 4x mode).
  - Abs/Square/Sqrt all live in one ACT table set - no table reloads.
"""

import sys
import types

for _p in ("/opt/trn_rl_repo", "/root/.axon_site"):
    if _p not in sys.path:
        sys.path.insert(0, _p)

# The NTFF profiling hook module is absent from this image's antenv package;
# inject the ctypes-based equivalent so trace=True works when requested.
if "antenv.axon_hooks" not in sys.modules:
    try:
        import trn_agent_boot.trn_boot as _tb

        _hook = _tb._ntff_profile_via_ctypes("/opt/axon/libaxon_pjrt.so")
        _m = types.ModuleType("antenv.axon_hooks")
        _m.get_axon_ntff_profile_hook = lambda: _hook
        sys.modules["antenv.axon_hooks"] = _m
    except Exception:
        pass

import numpy as np

import concourse.bacc as bacc
import concourse.tile as tile
import concourse.mybir as mybir
from concourse.bass_utils import run_bass_kernel_spmd

F32 = mybir.dt.float32
BF16 = mybir.dt.bfloat16
FP8 = mybir.dt.float8e4
ALU = mybir.AluOpType
ACTF = mybir.ActivationFunctionType
DR = mybir.MatmulPerfMode.DoubleRow

N_CORES = 8
B, C, D = 8192, 10000, 512
BPC = B // N_CORES          # 1024 batch rows per core
NB = BPC // 128             # 8 batch tiles
ND = D // 128               # 4 contraction sub-tiles
SLC = C // N_CORES          # 1250 classes whose norms this core owns
SEGW = 1000                 # prototype segment width
NSEG = C // SEGW            # 10
GW = 2000                   # main GEMM/ACT column group (4 PSUM banks)
NG = C // GW                # 5
OBW = 2500                  # affine/store chunk
NOB = C // OBW              # 4
WIN = 3                     # batch tiles interleaved with prep

# normalize engine per segment: DVE is ~1.6x faster per op than GPSIMD at
# f32-input tensor_tensor, so DVE takes 6 segments and GPSIMD 4.
NORM_ENG = {s: ("gps" if s in (1, 3, 5, 7) else "dve") for s in range(NSEG)}


def build_nc():
    nc = bacc.Bacc("TRN2", target_bir_lowering=False, debug=False,
                   num_devices=N_CORES)
    ft_h = nc.dram_tensor("ft", [D, BPC], F32, kind="ExternalInput")
    pt_h = nc.dram_tensor("pt", [D, C], F32, kind="ExternalInput")
    ptsl_h = nc.dram_tensor("ptsl", [D, SLC], F32, kind="ExternalInput")
    s_h = nc.dram_tensor("s", [1, 2], F32, kind="ExternalInput")
    o_h = nc.dram_tensor("o", [BPC, C], BF16, kind="ExternalOutput")

    from contextlib import ExitStack

    with tile.TileContext(nc) as tc:
        with ExitStack() as stack:
            ep = stack.enter_context
            const_pool = ep(tc.tile_pool(name="const", bufs=1))
            persist_pool = ep(tc.tile_pool(name="persist", bufs=1))
            stage_pool = ep(tc.tile_pool(name="stage", bufs=3))
            sq_pool = ep(tc.tile_pool(name="sq", bufs=2))
            row_pool = ep(tc.tile_pool(name="rows", bufs=1))
            nbb_pool = ep(tc.tile_pool(name="nbb", bufs=2))
            dq_pool = ep(tc.tile_pool(name="dq", bufs=WIN))
            rs_pool = ep(tc.tile_pool(name="rs", bufs=WIN))
            tail_pool = ep(tc.tile_pool(name="tail", bufs=2))
            ob_pool = ep(tc.tile_pool(name="ob", bufs=2))
            dram_pool = ep(tc.tile_pool(name="dram", bufs=1, space="DRAM"))
            # single shared PSUM pool, 2 x [128,4,512] f32 = all 8 banks;
            # every allocation is short-lived so rotation never blocks.
            ps_pool = ep(tc.tile_pool(name="ps", bufs=2, space="PSUM"))

            pnT = persist_pool.tile([128, ND, C], FP8, tag="pnT")
            fnT = persist_pool.tile([128, ND, BPC], FP8, tag="fnT")
            cb = persist_pool.tile([128, 2], F32, tag="cb")  # c0, c1

            ones_f = const_pool.tile([1, 128], F32, tag="ones_f")
            nc.vector.memset(ones_f[:, :], 1.0)
            ones_row = const_pool.tile([1, 128], BF16, tag="ones_row")
            nc.vector.memset(ones_row[:, :], 1.0)
            ones_col = const_pool.tile([128, 1], BF16, tag="ones_col")
            nc.vector.memset(ones_col[:, :], 1.0)

            # ---- scalar params: c0 = -|ds|/temp, c1 = c0/C ----------------
            stile = const_pool.tile([1, 2], F32, tag="stile")
            nc.sync.dma_start(stile[:, :], s_h[:, :])
            cv = const_pool.tile([1, 2], F32, tag="cvals")
            tmp = const_pool.tile([1, 2], F32, tag="scaltmp")
            nc.scalar.activation(tmp[:, 0:1], stile[:, 0:1], ACTF.Abs)
            nc.vector.reciprocal(tmp[:, 1:2], stile[:, 1:2])
            nc.vector.scalar_tensor_tensor(cv[:, 0:1], tmp[:, 0:1], -1.0,
                                           tmp[:, 1:2], op0=ALU.mult,
                                           op1=ALU.mult)
            nc.vector.tensor_scalar(cv[:, 1:2], cv[:, 0:1], 1.0 / C, None,
                                    op0=ALU.mult)
            ps_b = ps_pool.tile([128, 4, 512], F32, tag="ps", name="cbb")
            nc.tensor.matmul(ps_b[:, 0, :2], ones_f[:, :], cv[:, :],
                             start=True, stop=True)
            nc.vector.tensor_copy(cb[:, :], ps_b[:, 0, :2])

            ft_r = ft_h[:, :].rearrange("(t p) b -> p t b", p=128)
            pt_r = pt_h[:, :].rearrange("(t p) c -> p t c", p=128)
            ptsl_r = ptsl_h[:, :].rearrange("(t p) c -> p t c", p=128)

            # ---- features: load, local norms, normalize to fp8 ------------
            fstage = stage_pool.tile([128, ND, 1024], F32, tag="stg",
                                     name="fstage")
            nc.sync.dma_start(fstage[:, :, :], ft_r[:, :, :])
            sqf = sq_pool.tile([128, ND, 1024], BF16, tag="sq", name="sqf")
            nc.vector.tensor_tensor(sqf[:, :, :], fstage[:, :, :],
                                    fstage[:, :, :], op=ALU.mult)
            psf = ps_pool.tile([128, 4, 512], F32, tag="ps", name="psf")
            for h in range(2):
                for d in range(ND):
                    nc.tensor.matmul(psf[0:1, h, :512], ones_col[:, :],
                                     sqf[:, d, h * 512:(h + 1) * 512],
                                     start=(d == 0), stop=(d == ND - 1))
            firow = row_pool.tile([1, 2, 512], F32, tag="irow", name="firow")
            nc.vector.reciprocal_approx_fast(firow[:, :, :], psf[0:1, 0:2, :])
            fbrow = row_pool.tile([1, 2, 512], BF16, tag="brow", name="fbrow")
            nc.vector.tensor_copy(fbrow[:, :, :], firow[:, :, :])
            for h in range(2):
                nc.tensor.matmul(psf[:, 2 + h, :512], ones_row[:, :],
                                 fbrow[:, h, :], start=True, stop=True)
            fnb = nbb_pool.tile([128, 2, 512], BF16, tag="nbb", name="fnb")
            nc.scalar.activation(fnb[:, :, :], psf[:, 2:4, :], ACTF.Sqrt)
            for d in range(ND):
                nc.vector.tensor_tensor(
                    fnT[:, d, :].rearrange("p (h c) -> p h c", h=2),
                    fstage[:, d, :].rearrange("p (h c) -> p h c", h=2),
                    fnb[:, :, :], op=ALU.mult)

            # ---- this core's 1250-class inv-norm^2 slice + AllGather ------
            # slice staged as 1024 + 226 columns through the regular pools.
            cc_in = dram_pool.tile([1, SLC], F32, tag="cc_in")
            cc_out = dram_pool.tile([1, C], F32, tag="cc_out")
            psl_a = stage_pool.tile([128, ND, 1024], F32, tag="stg",
                                    name="psl_a")
            nc.sync.dma_start(psl_a[:, :, :], ptsl_r[:, :, 0:1024])
            psl_b = stage_pool.tile([128, ND, 1024], F32, tag="stg",
                                    name="psl_b")
            nc.sync.dma_start(psl_b[:, :, :SLC - 1024],
                              ptsl_r[:, :, 1024:SLC])
            sq_a = sq_pool.tile([128, ND, 1024], BF16, tag="sq", name="sq_a")
            nc.vector.tensor_tensor(sq_a[:, :, :], psl_a[:, :, :],
                                    psl_a[:, :, :], op=ALU.mult)
            sq_b = sq_pool.tile([128, ND, 1024], BF16, tag="sq", name="sq_b")
            nc.vector.tensor_tensor(sq_b[:, :, :SLC - 1024],
                                    psl_b[:, :, :SLC - 1024],
                                    psl_b[:, :, :SLC - 1024], op=ALU.mult)
            pss = ps_pool.tile([128, 4, 512], F32, tag="ps", name="pss")
            for h in range(2):
                for d in range(ND):
                    nc.tensor.matmul(pss[0:1, h, :512], ones_col[:, :],
                                     sq_a[:, d, h * 512:(h + 1) * 512],
                                     start=(d == 0), stop=(d == ND - 1))
            for d in range(ND):
                nc.tensor.matmul(pss[0:1, 2, :SLC - 1024], ones_col[:, :],
                                 sq_b[:, d, :SLC - 1024],
                                 start=(d == 0), stop=(d == ND - 1))
            sirow = row_pool.tile([1, 3, 512], F32, tag="sirow", name="sirow")
            nc.vector.reciprocal_approx_fast(sirow[:, 0:2, :],
                                             pss[0:1, 0:2, :])
            nc.vector.reciprocal_approx_fast(sirow[:, 2, :SLC - 1024],
                                             pss[0:1, 2, :SLC - 1024])
            nc.sync.dma_start(
                cc_in[:, 0:1024],
                sirow[:, 0:2, :].rearrange("o h c -> o (h c)"))
            nc.sync.dma_start(cc_in[:, 1024:SLC], sirow[:, 2, :SLC - 1024])
            nc.gpsimd.collective_compute(
                "AllGather", ALU.bypass,
                replica_groups=[list(range(N_CORES))],
                ins=[cc_in[:, :]], outs=[cc_out[:, :]])

            # ---- pipelined prototype segments + interleaved main groups ---
            def load(s):
                pst = stage_pool.tile([128, ND, 1024], F32, tag="stg",
                                      name=f"pst_{s}")
                nc.sync.dma_start(pst[:, :, :SEGW],
                                  pt_r[:, :, s * SEGW:(s + 1) * SEGW])
                return pst

            def invrow(s):
                """Fetch gathered inv-norm^2 cols [s*SEGW,(s+1)*SEGW), make
                the [128, 2, 500] bf16 inv-norm broadcast in SBUF."""
                grow = row_pool.tile([1, 2, 512], F32, tag="grow",
                                     name=f"grow_{s}")
                nc.sync.dma_start(
                    grow[:, :, :500],
                    cc_out[:, s * SEGW:(s + 1) * SEGW].rearrange(
                        "o (h c) -> o h c", h=2))
                brow = row_pool.tile([1, 2, 512], BF16, tag="brow",
                                     name=f"brow_{s}")
                nc.vector.tensor_copy(brow[:, :, :500], grow[:, :, :500])
                ps = ps_pool.tile([128, 4, 512], F32, tag="ps",
                                  name=f"psb_{s}")
                for h in range(2):
                    nc.tensor.matmul(ps[:, 2 + h, :500], ones_row[:, :],
                                     brow[:, h, :500], start=True, stop=True)
                nbb = nbb_pool.tile([128, 2, 512], BF16, tag="nbb",
                                    name=f"nbb_{s}")
                nc.scalar.activation(nbb[:, :, :500], ps[:, 2:4, :500],
                                     ACTF.Sqrt)
                return nbb

            def normalize(s, pst, nbb):
                eng = nc.vector if NORM_ENG[s] == "dve" else nc.gpsimd
                for d in range(ND):
                    eng.tensor_tensor(
                        pnT[:, d, s * SEGW:(s + 1) * SEGW].rearrange(
                            "p (h c) -> p h c", h=2),
                        pst[:, d, :SEGW].rearrange("p (h c) -> p h c", h=2),
                        nbb[:, :, :500], op=ALU.mult)

            def main_group(t, g, rs, dq):
                """DR GEMM + fused sqrt for cols [g*GW, (g+1)*GW), tile t."""
                pm = ps_pool.tile([128, 4, 512], F32, tag="ps",
                                  name=f"pm_{t}_{g}")
                for sub in range(4):
                    c0 = g * GW + sub * 500
                    for dp in range(ND // 2):
                        nc.tensor.matmul(
                            pm[:, sub, :500],
                            fnT[:, 2 * dp:2 * dp + 2,
                                t * 128:(t + 1) * 128],
                            pnT[:, 2 * dp:2 * dp + 2, c0:c0 + 500],
                            start=(dp == 0), stop=(dp == ND // 2 - 1),
                            perf_mode=DR)
                nc.scalar.activation(
                    dq[:, g * GW:(g + 1) * GW].rearrange(
                        "p (a c) -> p a c", a=4),
                    pm[:, :, :500], ACTF.Sqrt, bias=1.0, scale=-1.0,
                    accum_out=rs[:, g:g + 1])

            def tail(t, rs, dq):
                rsum = tail_pool.tile([128, 1], F32, tag="rsum",
                                      name=f"rsum_{t}")
                bvec = tail_pool.tile([128, 1], F32, tag="bvec",
                                      name=f"bvec_{t}")
                nc.vector.reduce_sum(rsum[:, :], rs[:, :],
                                     axis=mybir.AxisListType.X)
                nc.vector.tensor_scalar(bvec[:, :], rsum[:, :], cb[:, 1:2],
                                        None, op0=ALU.mult)
                for q in range(NOB):
                    ob = ob_pool.tile([128, OBW], BF16, tag="ob",
                                      name=f"ob_{t}_{q}")
                    nc.vector.tensor_scalar(ob[:, :],
                                            dq[:, q * OBW:(q + 1) * OBW],
                                            cb[:, 0:1], bvec[:, 0:1],
                                            op0=ALU.mult, op1=ALU.add)
                    nc.sync.dma_start(
                        o_h[t * 128:(t + 1) * 128, q * OBW:(q + 1) * OBW],
                        ob[:, :])

            # groups (t, g) scheduled at round 2g+2 (t0, t1) and 2g+3 (t2)
            SCHED = {r: [] for r in range(NSEG)}
            for g in range(NG - 1):
                SCHED[2 * g + 2] += [(0, g), (1, g)]
                if 2 * g + 3 < NSEG:
                    SCHED[2 * g + 3] += [(2, g)]

            dqs = {t: dq_pool.tile([128, C], BF16, tag="dq", name=f"dq_{t}")
                   for t in range(WIN)}
            rss = {t: rs_pool.tile([128, NG], F32, tag="rs", name=f"rs_{t}")
                   for t in range(WIN)}

            psts = {}
            nbbs = {}
            psts[0] = load(0)
            psts[1] = load(1)
            nbbs[0] = invrow(0)
            for r in range(NSEG):
                if r + 2 < NSEG:
                    psts[r + 2] = load(r + 2)
                if r + 1 < NSEG:
                    nbbs[r + 1] = invrow(r + 1)
                for (t, g) in SCHED[r]:
                    main_group(t, g, rss[t], dqs[t])
                normalize(r, psts[r], nbbs[r])
            for t in range(WIN):
                main_group(t, NG - 1, rss[t], dqs[t])
            for t in range(WIN):
                tail(t, rss[t], dqs[t])
            for t in range(WIN, NB):
                rs = rs_pool.tile([128, NG], F32, tag="rs", name=f"rs_{t}")
                dq = dq_pool.tile([128, C], BF16, tag="dq", name=f"dq_{t}")
                for g in range(NG):
                    main_group(t, g, rs, dq)
                tail(t, rs, dq)

    nc.compile()
    return nc


_CACHE = {}


def _get_nc():
    if "nc" not in _CACHE:
        _CACHE["nc"] = build_nc()
    return _CACHE["nc"]


def make_in_maps(features, prototypes, distance_scale, temperature):
    f = np.asarray(features, dtype=np.float32)
    ft = np.ascontiguousarray(f.T)              # [D, B]
    pt = np.ascontiguousarray(
        np.asarray(prototypes, dtype=np.float32).T)  # [D, C]
    s = np.array([[np.float32(np.asarray(distance_scale).reshape(-1)[0]),
                   np.float32(np.asarray(temperature).reshape(-1)[0])]],
                 dtype=np.float32)
    return [
        {"ft": np.ascontiguousarray(ft[:, i * BPC:(i + 1) * BPC]),
         "pt": pt,
         "ptsl": np.ascontiguousarray(pt[:, i * SLC:(i + 1) * SLC]),
         "s": s}
        for i in range(N_CORES)
    ]


def run(features, prototypes, distance_scale, temperature, **kwargs):
    nc = _get_nc()
    in_maps = make_in_maps(features, prototypes, distance_scale, temperature)
    res = run_bass_kernel_spmd(nc, in_maps, core_ids=list(range(N_CORES)),
                               **kwargs)
    out = np.concatenate(
        [np.asarray(res.results[i]["o"]).astype(np.float32)
         for i in range(N_CORES)], axis=0)
    return out, res


def kernel(features, prototypes, distance_scale, temperature):
    out, _ = run(features, prototypes, distance_scale, temperature)
    return out


# revision 14
# speedup vs baseline: 1.1163x; 1.1163x over previous
"""DisMax loss first part: logits = -(|s|*d + mean_c(|s|*d)) / temp, where
d[b,c] = ||fn_b - pn_c|| / sqrt(2) = sqrt(1 - cos(f_b, p_c)) for l2-normalized rows.

Data-parallel over the batch across 8 NeuronCores; each core computes
[1024, 512] x [512, 10000] with all numerics on device (host does layout
transposes and the final bf16->f32 upcast only).

v5 (local norms, fully software-pipelined; v4's 8-core AllGather of the
norm rows cost ~45us of dead latency and was dropped):
  - Per segment: square (GPSIMD mostly - one ~7us job per round), bf16
    ones-reduce matmuls, DVE reciprocal_approx_fast on the [1,1000] row,
    bf16 cast, PE ones-broadcast of inv-norm^2, one fused ACT Sqrt-copy
    (PSUM->SBUF) -> [128,1000] inv-norm operand, then the DVE/GPSIMD
    normalize writes fp8 pnT.
  - Main GEMM: fp8 DoubleRow, 2000-col PSUM groups; groups lag their
    segments by a full round so the in-order PE queue never head-of-line
    blocks on a normalize; 3-tile window overlaps main work with prep.
  - Final affine on DVE tensor_scalar (bf16 4x mode, measured 930ns per
    [128,2500] vs 2433ns on GPSIMD); two tiles' affines go to GPSIMD
    which is idle post-prep.
  - Every PSUM tile is short-lived inside one rotating 2 x [128,4,512]
    pool (8 banks); group allocations precede the norm-chain allocation
    each round to keep the 2-deep rotation deadlock-free.
  - Abs/Square/Sqrt all live in one ACT table set - no table reloads.
"""

import sys
import types

for _p in ("/opt/trn_rl_repo", "/root/.axon_site"):
    if _p not in sys.path:
        sys.path.insert(0, _p)

# The NTFF profiling hook module is absent from this image's antenv package;
# inject the ctypes-based equivalent so trace=True works when requested.
if "antenv.axon_hooks" not in sys.modules:
    try:
        import trn_agent_boot.trn_boot as _tb

        _hook = _tb._ntff_profile_via_ctypes("/opt/axon/libaxon_pjrt.so")
        _m = types.ModuleType("antenv.axon_hooks")
        _m.get_axon_ntff_profile_hook = lambda: _hook
        sys.modules["antenv.axon_hooks"] = _m
    except Exception:
        pass

import numpy as np

import concourse.bacc as bacc
import concourse.tile as tile
import concourse.mybir as mybir
from concourse.bass_utils import run_bass_kernel_spmd

F32 = mybir.dt.float32
BF16 = mybir.dt.bfloat16
FP8 = mybir.dt.float8e4
ALU = mybir.AluOpType
ACTF = mybir.ActivationFunctionType
DR = mybir.MatmulPerfMode.DoubleRow

N_CORES = 8
B, C, D = 8192, 10000, 512
BPC = B // N_CORES          # 1024 batch rows per core
NB = BPC // 128             # 8 batch tiles
ND = D // 128               # 4 contraction sub-tiles
SEGW = 1000                 # prototype segment width
NSEG = C // SEGW            # 10
GW = 2000                   # main GEMM/ACT column group (4 PSUM banks)
NG = C // GW                # 5
OBW = 2500                  # affine/store chunk
NOB = C // OBW              # 4
WIN = 3                     # batch tiles interleaved with prep

SQ_ENG = {0: "dve", 1: "dve", 2: "gps", 3: "act", 4: "gps",
          5: "act", 6: "gps", 7: "act", 8: "gps", 9: "gps"}
NORM_ENG = {s: ("gps" if s in (3, 7) else "dve") for s in range(NSEG)}
AFF_ENG = {t: ("gps" if t in (5, 6) else "dve") for t in range(NB)}


def build_nc():
    nc = bacc.Bacc("TRN2", target_bir_lowering=False, debug=False,
                   num_devices=N_CORES)
    ft_h = nc.dram_tensor("ft", [D, BPC], F32, kind="ExternalInput")
    pt_h = nc.dram_tensor("pt", [D, C], F32, kind="ExternalInput")
    s_h = nc.dram_tensor("s", [1, 2], F32, kind="ExternalInput")
    o_h = nc.dram_tensor("o", [BPC, C], BF16, kind="ExternalOutput")

    from contextlib import ExitStack

    with tile.TileContext(nc) as tc:
        with ExitStack() as stack:
            ep = stack.enter_context
            const_pool = ep(tc.tile_pool(name="const", bufs=1))
            persist_pool = ep(tc.tile_pool(name="persist", bufs=1))
            stage_pool = ep(tc.tile_pool(name="stage", bufs=3))
            sq_pool = ep(tc.tile_pool(name="sq", bufs=2))
            row_pool = ep(tc.tile_pool(name="rows", bufs=1))
            nbb_pool = ep(tc.tile_pool(name="nbb", bufs=2))
            dq_pool = ep(tc.tile_pool(name="dq", bufs=WIN))
            rs_pool = ep(tc.tile_pool(name="rs", bufs=WIN))
            tail_pool = ep(tc.tile_pool(name="tail", bufs=2))
            ob_pool = ep(tc.tile_pool(name="ob", bufs=3))
            # single shared PSUM pool, 2 x [128,4,512] f32 = all 8 banks;
            # every allocation is short-lived so rotation never blocks.
            ps_pool = ep(tc.tile_pool(name="ps", bufs=2, space="PSUM"))

            pnT = persist_pool.tile([128, ND, C], FP8, tag="pnT")
            fnT = persist_pool.tile([128, ND, BPC], FP8, tag="fnT")
            cb = persist_pool.tile([128, 2], F32, tag="cb")  # c0, c1

            ones_f = const_pool.tile([1, 128], F32, tag="ones_f")
            nc.vector.memset(ones_f[:, :], 1.0)
            ones_row = const_pool.tile([1, 128], BF16, tag="ones_row")
            nc.vector.memset(ones_row[:, :], 1.0)
            ones_col = const_pool.tile([128, 1], BF16, tag="ones_col")
            nc.vector.memset(ones_col[:, :], 1.0)

            # ---- scalar params: c0 = -|ds|/temp, c1 = c0/C ----------------
            stile = const_pool.tile([1, 2], F32, tag="stile")
            nc.sync.dma_start(stile[:, :], s_h[:, :])
            cv = const_pool.tile([1, 2], F32, tag="cvals")
            tmp = const_pool.tile([1, 2], F32, tag="scaltmp")
            nc.scalar.activation(tmp[:, 0:1], stile[:, 0:1], ACTF.Abs)
            nc.vector.reciprocal(tmp[:, 1:2], stile[:, 1:2])
            nc.vector.scalar_tensor_tensor(cv[:, 0:1], tmp[:, 0:1], -1.0,
                                           tmp[:, 1:2], op0=ALU.mult,
                                           op1=ALU.mult)
            nc.vector.tensor_scalar(cv[:, 1:2], cv[:, 0:1], 1.0 / C, None,
                                    op0=ALU.mult)
            ps_b = ps_pool.tile([128, 4, 512], F32, tag="ps", name="cbb")
            nc.tensor.matmul(ps_b[:, 0, :2], ones_f[:, :], cv[:, :],
                             start=True, stop=True)
            nc.vector.tensor_copy(cb[:, :], ps_b[:, 0, :2])

            ft_r = ft_h[:, :].rearrange("(t p) b -> p t b", p=128)
            pt_r = pt_h[:, :].rearrange("(t p) c -> p t c", p=128)

            # ---- features: load, norms, normalize to fp8 ------------------
            fstage = stage_pool.tile([128, ND, 1024], F32, tag="stg",
                                     name="fstage")
            nc.sync.dma_start(fstage[:, :, :], ft_r[:, :, :])
            sqf = sq_pool.tile([128, ND, 1024], BF16, tag="sq", name="sqf")
            nc.gpsimd.tensor_tensor(sqf[:, :, :], fstage[:, :, :],
                                    fstage[:, :, :], op=ALU.mult)
            psf = ps_pool.tile([128, 4, 512], F32, tag="ps", name="psf")
            for h in range(2):
                for d in range(ND):
                    nc.tensor.matmul(psf[0:1, h, :512], ones_col[:, :],
                                     sqf[:, d, h * 512:(h + 1) * 512],
                                     start=(d == 0), stop=(d == ND - 1))
            firow = row_pool.tile([1, 2, 512], F32, tag="irow", name="firow")
            nc.vector.reciprocal_approx_fast(firow[:, :, :], psf[0:1, 0:2, :])
            fbrow = row_pool.tile([1, 2, 512], BF16, tag="brow", name="fbrow")
            nc.vector.tensor_copy(fbrow[:, :, :], firow[:, :, :])
            for h in range(2):
                nc.tensor.matmul(psf[:, 2 + h, :512], ones_row[:, :],
                                 fbrow[:, h, :], start=True, stop=True)
            fnb = nbb_pool.tile([128, 2, 512], BF16, tag="nbb", name="fnb")
            nc.scalar.activation(fnb[:, :, :], psf[:, 2:4, :], ACTF.Sqrt)
            for d in range(ND):
                nc.vector.tensor_tensor(
                    fnT[:, d, :].rearrange("p (h c) -> p h c", h=2),
                    fstage[:, d, :].rearrange("p (h c) -> p h c", h=2),
                    fnb[:, :, :], op=ALU.mult)

            # ---- pipelined prototype segments + interleaved main groups ---
            def load(s):
                pst = stage_pool.tile([128, ND, 1024], F32, tag="stg",
                                      name=f"pst_{s}")
                nc.sync.dma_start(pst[:, :, :SEGW],
                                  pt_r[:, :, s * SEGW:(s + 1) * SEGW])
                return pst

            def square(s, pst):
                sqt = sq_pool.tile([128, ND, 1024], BF16, tag="sq",
                                   name=f"sq_{s}")
                eng = {"dve": nc.vector, "gps": nc.gpsimd}.get(SQ_ENG[s])
                if eng is None:
                    nc.scalar.activation(sqt[:, :, :SEGW], pst[:, :, :SEGW],
                                         ACTF.Square)
                else:
                    eng.tensor_tensor(sqt[:, :, :SEGW], pst[:, :, :SEGW],
                                      pst[:, :, :SEGW], op=ALU.mult)
                return sqt

            def invrow_a(s, sqt):
                """reduce rows + reciprocal + bf16 cast; returns (ps, brow)."""
                ps = ps_pool.tile([128, 4, 512], F32, tag="ps",
                                  name=f"psn_{s}")
                for h in range(2):
                    for d in range(ND):
                        nc.tensor.matmul(ps[0:1, h, :500], ones_col[:, :],
                                         sqt[:, d, h * 500:(h + 1) * 500],
                                         start=(d == 0), stop=(d == ND - 1))
                irow = row_pool.tile([1, 2, 512], F32, tag="irow",
                                     name=f"irow_{s}")
                nc.vector.reciprocal_approx_fast(irow[:, :, :500],
                                                 ps[0:1, 0:2, :500])
                brow = row_pool.tile([1, 2, 512], BF16, tag="brow",
                                     name=f"brow_{s}")
                nc.vector.tensor_copy(brow[:, :, :500], irow[:, :, :500])
                return ps, brow

            def invrow_b(s, ps, brow):
                """broadcast inv-norm^2 + fused ACT Sqrt-copy to SBUF."""
                for h in range(2):
                    nc.tensor.matmul(ps[:, 2 + h, :500], ones_row[:, :],
                                     brow[:, h, :500], start=True, stop=True)
                nbb = nbb_pool.tile([128, 2, 512], BF16, tag="nbb",
                                    name=f"nbb_{s}")
                nc.scalar.activation(nbb[:, :, :500], ps[:, 2:4, :500],
                                     ACTF.Sqrt)
                return nbb

            def normalize(s, pst, nbb):
                eng = nc.vector if NORM_ENG[s] == "dve" else nc.gpsimd
                for d in range(ND):
                    eng.tensor_tensor(
                        pnT[:, d, s * SEGW:(s + 1) * SEGW].rearrange(
                            "p (h c) -> p h c", h=2),
                        pst[:, d, :SEGW].rearrange("p (h c) -> p h c", h=2),
                        nbb[:, :, :500], op=ALU.mult)

            def main_group(t, g, rs, dq):
                """DR GEMM + fused sqrt for cols [g*GW, (g+1)*GW), tile t."""
                pm = ps_pool.tile([128, 4, 512], F32, tag="ps",
                                  name=f"pm_{t}_{g}")
                for sub in range(4):
                    c0 = g * GW + sub * 500
                    for dp in range(ND // 2):
                        nc.tensor.matmul(
                            pm[:, sub, :500],
                            fnT[:, 2 * dp:2 * dp + 2,
                                t * 128:(t + 1) * 128],
                            pnT[:, 2 * dp:2 * dp + 2, c0:c0 + 500],
                            start=(dp == 0), stop=(dp == ND // 2 - 1),
                            perf_mode=DR)
                nc.scalar.activation(
                    dq[:, g * GW:(g + 1) * GW].rearrange(
                        "p (a c) -> p a c", a=4),
                    pm[:, :, :500], ACTF.Sqrt, bias=1.0, scale=-1.0,
                    accum_out=rs[:, g:g + 1])

            def tail(t, rs, dq):
                rsum = tail_pool.tile([128, 1], F32, tag="rsum",
                                      name=f"rsum_{t}")
                bvec = tail_pool.tile([128, 1], F32, tag="bvec",
                                      name=f"bvec_{t}")
                nc.vector.reduce_sum(rsum[:, :], rs[:, :],
                                     axis=mybir.AxisListType.X)
                nc.vector.tensor_scalar(bvec[:, :], rsum[:, :], cb[:, 1:2],
                                        None, op0=ALU.mult)
                eng = nc.vector if AFF_ENG[t] == "dve" else nc.gpsimd
                for q in range(NOB):
                    ob = ob_pool.tile([128, OBW], BF16, tag="ob",
                                      name=f"ob_{t}_{q}")
                    eng.tensor_scalar(ob[:, :],
                                      dq[:, q * OBW:(q + 1) * OBW],
                                      cb[:, 0:1], bvec[:, 0:1],
                                      op0=ALU.mult, op1=ALU.add)
                    nc.sync.dma_start(
                        o_h[t * 128:(t + 1) * 128, q * OBW:(q + 1) * OBW],
                        ob[:, :])

            # groups (t, g) scheduled at round 2g+2 (t0, t1) and 2g+3 (t2)
            SCHED = {r: [] for r in range(NSEG)}
            for g in range(NG - 1):
                SCHED[2 * g + 2] += [(0, g), (1, g)]
                if 2 * g + 3 < NSEG:
                    SCHED[2 * g + 3] += [(2, g)]

            dqs = {t: dq_pool.tile([128, C], BF16, tag="dq", name=f"dq_{t}")
                   for t in range(WIN)}
            rss = {t: rs_pool.tile([128, NG], F32, tag="rs", name=f"rs_{t}")
                   for t in range(WIN)}

            psts = {}
            sqs = {}
            nbbs = {}
            psts[0] = load(0)
            psts[1] = load(1)
            sqs[0] = square(0, psts[0])
            sqs[1] = square(1, psts[1])
            ps0, brow0 = invrow_a(0, sqs[0])
            nbbs[0] = invrow_b(0, ps0, brow0)
            for r in range(NSEG):
                if r + 2 < NSEG:
                    psts[r + 2] = load(r + 2)
                    sqs[r + 2] = square(r + 2, psts[r + 2])
                for (t, g) in SCHED[r]:
                    main_group(t, g, rss[t], dqs[t])
                if r + 1 < NSEG:
                    ps_n, brow_n = invrow_a(r + 1, sqs[r + 1])
                    nbbs[r + 1] = invrow_b(r + 1, ps_n, brow_n)
                normalize(r, psts[r], nbbs[r])
            for t in range(WIN):
                main_group(t, NG - 1, rss[t], dqs[t])
            for t in range(WIN):
                tail(t, rss[t], dqs[t])
            for t in range(WIN, NB):
                rs = rs_pool.tile([128, NG], F32, tag="rs", name=f"rs_{t}")
                dq = dq_pool.tile([128, C], BF16, tag="dq", name=f"dq_{t}")
                for g in range(NG):
                    main_group(t, g, rs, dq)
                tail(t, rs, dq)

    nc.compile()
    return nc


_CACHE = {}


def _get_nc():
    if "nc" not in _CACHE:
        _CACHE["nc"] = build_nc()
    return _CACHE["nc"]


def make_in_maps(features, prototypes, distance_scale, temperature):
    f = np.asarray(features, dtype=np.float32)
    ft = np.ascontiguousarray(f.T)              # [D, B]
    pt = np.ascontiguousarray(
        np.asarray(prototypes, dtype=np.float32).T)  # [D, C]
    s = np.array([[np.float32(np.asarray(distance_scale).reshape(-1)[0]),
                   np.float32(np.asarray(temperature).reshape(-1)[0])]],
                 dtype=np.float32)
    return [
        {"ft": np.ascontiguousarray(ft[:, i * BPC:(i + 1) * BPC]),
         "pt": pt, "s": s}
        for i in range(N_CORES)
    ]


def run(features, prototypes, distance_scale, temperature, **kwargs):
    nc = _get_nc()
    in_maps = make_in_maps(features, prototypes, distance_scale, temperature)
    res = run_bass_kernel_spmd(nc, in_maps, core_ids=list(range(N_CORES)),
                               **kwargs)
    out = np.concatenate(
        [np.asarray(res.results[i]["o"]).astype(np.float32)
         for i in range(N_CORES)], axis=0)
    return out, res


def kernel(features, prototypes, distance_scale, temperature):
    out, _ = run(features, prototypes, distance_scale, temperature)
    return out
